# revision 17
# baseline (speedup 1.0000x reference)
"""AttnDecoderRNN single-step kernel for 8 trn2 NeuronCores.

Strategy (SPMD, one graph, per-core data):
  - batch-parallel front: each core owns 128 of 1024 batch rows.
      embedding gather (indirect DMA from replicated table) -> GRU cell
      (fp32 matmuls + exact DVE/ACT gate math) -> attention over S=10
      encoder positions (DVE fp32).
  - AllGather of the transposed hidden state (bf16) so every core holds
      hT for the full batch.
  - vocab-parallel back: each core owns a 6656-wide padded slice of the
      50257 vocab. logits = hT.T @ out_wT_shard (bf16 matmuls, fp32
      accum), +out_b via K=1 matmul, exp+row-sum fused on ScalarE,
      4KB AllReduce for the global softmax denominator, then
      out = logit - log(sum) written per chunk.
Host only reshapes/transposes/shards/pads and concatenates results.
"""

import os

os.environ.setdefault("MYCRO_LOCAL_CACHE", "1")

import numpy as np
import ml_dtypes

import concourse.bass as bass
import concourse.mybir as mybir
import concourse.tile as tile
from concourse import bacc
from concourse.bass_utils import run_bass_kernel_spmd
from concourse.masks import make_identity

F32 = mybir.dt.float32
F32R = mybir.dt.float32r
BF16 = mybir.dt.bfloat16
I32 = mybir.dt.int32
AX = mybir.AxisListType.X
ALU = mybir.AluOpType
ACTF = mybir.ActivationFunctionType

V = 50257
H = 1024
B = 1024
S = 10
NCORES = 8
BL = B // NCORES          # 128 batch rows per core
VCHUNK = 512              # logits n-chunk (one PSUM bank)
VP = 6656                 # padded vocab per core (13 * 512)
NVC = VP // VCHUNK        # 13
KT = H // 128             # 8 contraction tiles
G3 = 3 * H                # 3072
NGC = G3 // 512           # 6 gate chunks

# matmul dtype knobs
GRU_RDT = F32             # plain fp32 (f32r needs rounded producers)


def build_graph(stage=99):
    """stage gates (debug bisect): 1=gather 2=+gru 3=+attn 4=+allgather
    5=+logitsA 6=full"""
    nc = bacc.Bacc(None, target_bir_lowering=False, debug=False)

    # ---- per-core parameters ----
    p_idx = nc.declare_dram_parameter("idx", [BL, 1], I32, isOutput=False)
    p_h0 = nc.declare_dram_parameter("h0", [BL, H], F32, isOutput=False)
    p_h0T = nc.declare_dram_parameter("h0T", [H, BL], F32, isOutput=False)
    p_enc = nc.declare_dram_parameter("enc", [BL, S * H], F32, isOutput=False)
    p_mask = nc.declare_dram_parameter("mask", [BL, S], F32, isOutput=False)
    p_maskneg = nc.declare_dram_parameter("maskneg", [BL, S], F32, isOutput=False)
    p_wihT = nc.declare_dram_parameter("w_ihT", [H, G3], F32, isOutput=False)
    p_whhT = nc.declare_dram_parameter("w_hhT", [H, G3], F32, isOutput=False)
    p_bih = nc.declare_dram_parameter("b_ih", [1, G3], F32, isOutput=False)
    p_bhh = nc.declare_dram_parameter("b_hh", [1, G3], F32, isOutput=False)
    p_emb = nc.declare_dram_parameter("emb", [V, H], F32, isOutput=False)
    p_owT = nc.declare_dram_parameter("out_wT", [H, VP], BF16, isOutput=False)
    p_ob = nc.declare_dram_parameter("out_b", [1, VP], BF16, isOutput=False)

    o_logp = nc.declare_dram_parameter("out_logp", [B, VP], F32, isOutput=True)
    o_h = nc.declare_dram_parameter("out_h", [BL, H], F32, isOutput=True)
    o_attn = nc.declare_dram_parameter("out_attn", [BL, S], F32, isOutput=True)

    rg = [list(range(NCORES))]

    with tile.TileContext(nc) as tc:
        with (
            tc.tile_pool(name="const", bufs=1) as constp,
            tc.tile_pool(name="persist", bufs=1) as persist,
            tc.tile_pool(name="dram", bufs=1, space="DRAM") as dram,
            tc.tile_pool(name="tpsum", bufs=2, space="PSUM") as tpsum,
            tc.tile_pool(name="gpsum", bufs=4, space="PSUM") as gpsum,
            tc.tile_pool(name="grupool", bufs=1) as grup,
        ):
            # ---------- constants ----------
            ident = constp.tile([128, 128], F32, tag="ident")
            make_identity(nc, ident[:])
            ones1 = constp.tile([1, 128], F32, tag="ones1")
            nc.vector.memset(ones1[:], 1.0)
            ones1b = constp.tile([1, 128], BF16, tag="ones1b")
            nc.vector.memset(ones1b[:], 1.0)

            idx_sb = constp.tile([BL, 1], I32, tag="idx")
            nc.sync.dma_start(out=idx_sb[:], in_=p_idx[:, :])
            bih_sb = constp.tile([1, G3], F32, tag="bih")
            nc.sync.dma_start(out=bih_sb[:], in_=p_bih[:, :])
            bhh_sb = constp.tile([1, G3], F32, tag="bhh")
            nc.sync.dma_start(out=bhh_sb[:], in_=p_bhh[:, :])
            ob_sb = constp.tile([1, VP], BF16, tag="ob")
            nc.sync.dma_start(out=ob_sb[:], in_=p_ob[:, :])
            h0_sb = grup.tile([BL, H], F32, tag="h0")
            nc.sync.dma_start(out=h0_sb[:], in_=p_h0[:, :])
            h0T_sb = grup.tile([128, H], F32, tag="h0T")
            nc.sync.dma_start(
                out=h0T_sb[:].rearrange("p (k j) -> p k j", k=KT),
                in_=p_h0T[:, :].rearrange("(k p) j -> p k j", p=128),
            )
            mask_sb = constp.tile([BL, S], F32, tag="mask")
            nc.sync.dma_start(out=mask_sb[:], in_=p_mask[:, :])
            maskneg_sb = constp.tile([BL, S], F32, tag="maskneg")
            nc.sync.dma_start(out=maskneg_sb[:], in_=p_maskneg[:, :])

            # ---------- embedding gather + transpose ----------
            x_sb = grup.tile([BL, H], F32, tag="x")
            nc.gpsimd.indirect_dma_start(
                out=x_sb[:],
                out_offset=None,
                in_=p_emb[:, :],
                in_offset=bass.IndirectOffsetOnAxis(ap=idx_sb[:, :1], axis=0),
            )
            xT_sb = grup.tile([128, H], F32, tag="xT")
            for t in range(KT):
                tps = tpsum.tile([128, 128], F32, tag="tps")
                nc.tensor.transpose(tps[:], x_sb[:, t * 128 : (t + 1) * 128], ident[:])
                nc.vector.tensor_copy(out=xT_sb[:, t * 128 : (t + 1) * 128], in_=tps[:])
            if stage == 1:
                nc.sync.dma_start(out=o_h[:, :], in_=x_sb[:])

            # ---------- GRU ----------
            if stage >= 2:
                r_sb = grup.tile([BL, H], F32, tag="r")
                z_sb = grup.tile([BL, H], F32, tag="z")
                n_sb = grup.tile([BL, H], F32, tag="n")
                h_sb = persist.tile([BL, H], F32, tag="h")

                with (
                    tc.tile_pool(name="gw", bufs=2) as gwp,
                    tc.tile_pool(name="gtmp", bufs=3) as gtp,
                ):
                    for c in range(NGC):
                        cs = slice(c * 512, (c + 1) * 512)
                        wih = gwp.tile([128, KT * 512], F32, tag="wih")
                        nc.sync.dma_start(
                            out=wih[:].rearrange("p (k n) -> p k n", k=KT),
                            in_=p_wihT[:, cs].rearrange("(k p) n -> p k n", p=128),
                        )
                        whh = gwp.tile([128, KT * 512], F32, tag="whh")
                        nc.sync.dma_start(
                            out=whh[:].rearrange("p (k n) -> p k n", k=KT),
                            in_=p_whhT[:, cs].rearrange("(k p) n -> p k n", p=128),
                        )
                        if c < 4:
                            # r/z gates: gi+gh+biases in ONE psum group
                            ps = gpsum.tile([128, 512], F32, tag="mmps")
                            for k in range(KT):
                                nc.tensor.matmul(
                                    ps[:],
                                    lhsT=xT_sb[:, k * 128 : (k + 1) * 128].bitcast(GRU_RDT),
                                    rhs=wih[:, k * 512 : (k + 1) * 512].bitcast(GRU_RDT),
                                    start=(k == 0),
                                    stop=False,
                                )
                            for k in range(KT):
                                nc.tensor.matmul(
                                    ps[:],
                                    lhsT=h0T_sb[:, k * 128 : (k + 1) * 128].bitcast(GRU_RDT),
                                    rhs=whh[:, k * 512 : (k + 1) * 512].bitcast(GRU_RDT),
                                    start=False,
                                    stop=False,
                                )
                            nc.tensor.matmul(
                                ps[:], lhsT=ones1[:], rhs=bih_sb[:1, cs],
                                start=False, stop=False,
                            )
                            nc.tensor.matmul(
                                ps[:], lhsT=ones1[:], rhs=bhh_sb[:1, cs],
                                start=False, stop=True,
                            )
                            dst = r_sb if c < 2 else z_sb
                            ds_ = slice((c % 2) * 512, (c % 2) * 512 + 512)
                            nc.scalar.activation(out=dst[:, ds_], in_=ps[:], func=ACTF.Sigmoid)
                        else:
                            # n gate: tanh(gi + r * gh)
                            cc = c - 4
                            ncs = slice(cc * 512, cc * 512 + 512)
                            psi = gpsum.tile([128, 512], F32, tag="mmps")
                            psh = gpsum.tile([128, 512], F32, tag="mmps")
                            for k in range(KT):
                                nc.tensor.matmul(
                                    psi[:],
                                    lhsT=xT_sb[:, k * 128 : (k + 1) * 128].bitcast(GRU_RDT),
                                    rhs=wih[:, k * 512 : (k + 1) * 512].bitcast(GRU_RDT),
                                    start=(k == 0),
                                    stop=False,
                                )
                            nc.tensor.matmul(
                                psi[:], lhsT=ones1[:], rhs=bih_sb[:1, cs],
                                start=False, stop=True,
                            )
                            for k in range(KT):
                                nc.tensor.matmul(
                                    psh[:],
                                    lhsT=h0T_sb[:, k * 128 : (k + 1) * 128].bitcast(GRU_RDT),
                                    rhs=whh[:, k * 512 : (k + 1) * 512].bitcast(GRU_RDT),
                                    start=(k == 0),
                                    stop=False,
                                )
                            nc.tensor.matmul(
                                psh[:], lhsT=ones1[:], rhs=bhh_sb[:1, cs],
                                start=False, stop=True,
                            )
                            t1 = gtp.tile([128, 512], F32, tag="t1")
                            nc.vector.tensor_tensor(
                                out=t1[:], in0=r_sb[:, ncs], in1=psh[:], op=ALU.mult
                            )
                            t2 = gtp.tile([128, 512], F32, tag="t2")
                            nc.vector.tensor_tensor(
                                out=t2[:], in0=t1[:], in1=psi[:], op=ALU.add
                            )
                            nc.scalar.activation(out=n_sb[:, ncs], in_=t2[:], func=ACTF.Tanh)

                    # h = n + z * (h0 - n)
                    for q in range(2):
                        qs = slice(q * 512, (q + 1) * 512)
                        d = gtp.tile([128, 512], F32, tag="d")
                        nc.vector.tensor_tensor(
                            out=d[:], in0=h0_sb[:, qs], in1=n_sb[:, qs], op=ALU.subtract
                        )
                        e = gtp.tile([128, 512], F32, tag="e")
                        nc.vector.tensor_tensor(
                            out=e[:], in0=z_sb[:, qs], in1=d[:], op=ALU.mult
                        )
                        nc.vector.tensor_tensor(
                            out=h_sb[:, qs], in0=n_sb[:, qs], in1=e[:], op=ALU.add
                        )

                nc.sync.dma_start(out=o_h[:, :], in_=h_sb[:])

            # ---------- attention (DVE/ACT; overlaps logits PE work) ----------
            if stage >= 3:
                e_sb = persist.tile([BL, S], F32, tag="esb")
                with tc.tile_pool(name="encp", bufs=3) as encp:
                    for s in range(S):
                        encs = encp.tile([BL, H], F32, tag="encs")
                        nc.sync.dma_start(
                            out=encs[:], in_=p_enc[:, s * H : (s + 1) * H]
                        )
                        prod = encp.tile([BL, H], F32, tag="prod")
                        nc.vector.tensor_tensor(
                            out=prod[:], in0=h_sb[:], in1=encs[:], op=ALU.mult
                        )
                        nc.vector.reduce_sum(
                            out=e_sb[:, s : s + 1], in_=prod[:], axis=AX
                        )
                em = persist.tile([BL, S], F32, tag="em")
                nc.vector.tensor_tensor(out=em[:], in0=e_sb[:], in1=mask_sb[:], op=ALU.mult)
                emm = persist.tile([BL, S], F32, tag="emm")
                nc.vector.tensor_tensor(
                    out=emm[:], in0=em[:], in1=maskneg_sb[:], op=ALU.add
                )
                mx = persist.tile([BL, 1], F32, tag="mx")
                nc.vector.reduce_max(out=mx[:, :1], in_=emm[:], axis=AX)
                nmx = persist.tile([BL, 1], F32, tag="nmx")
                nc.vector.tensor_scalar_mul(out=nmx[:, :1], in0=mx[:, :1], scalar1=-1.0)
                ex = persist.tile([BL, S], F32, tag="ex")
                sume = persist.tile([BL, 1], F32, tag="sume")
                nc.scalar.activation(
                    out=ex[:], in_=emm[:], func=ACTF.Exp,
                    bias=nmx[:, :1], accum_out=sume[:, :1],
                )
                rcp = persist.tile([BL, 1], F32, tag="rcp")
                nc.vector.reciprocal(out=rcp[:, :1], in_=sume[:, :1])
                attn_sb = persist.tile([BL, S], F32, tag="attn")
                nc.vector.tensor_scalar_mul(out=attn_sb[:], in0=ex[:], scalar1=rcp[:, :1])
                nc.sync.dma_start(out=o_attn[:, :], in_=attn_sb[:])

            # ---------- hT AllGather ----------
            if stage >= 4:
                hTst = persist.tile([128, H], BF16, tag="hTst")
                for t in range(KT):
                    tps = tpsum.tile([128, 128], F32, tag="tps")
                    nc.tensor.transpose(tps[:], h_sb[:, t * 128 : (t + 1) * 128], ident[:])
                    nc.vector.tensor_copy(out=hTst[:, t * 128 : (t + 1) * 128], in_=tps[:])
                hT_bounce = dram.tile([H, BL], BF16, tag="hT_bounce")
                nc.sync.dma_start(
                    out=hT_bounce[:].rearrange("(t p) j -> p t j", p=128),
                    in_=hTst[:].rearrange("p (t j) -> p t j", t=KT),
                )
                hT_all = dram.tile([NCORES * H, BL], BF16, tag="hT_all", addr_space="Shared")
                nc.gpsimd.collective_compute(
                    "AllGather",
                    ALU.bypass,
                    ins=[hT_bounce[:].opt()],
                    outs=[hT_all[:].opt()],
                    replica_groups=rg,
                )
                hTall_sb = persist.tile([128, NCORES * H], BF16, tag="hTall")
                nc.sync.dma_start(
                    out=hTall_sb[:].rearrange("p (q j) -> p q j", q=NCORES * KT),
                    in_=hT_all[:].rearrange("(q p) j -> p q j", p=128),
                )

            # ---------- logits pass A: matmul, +bias, bf16 store, exp-sum ----------
            if stage >= 5:
                lg_dram = dram.tile([B, VP], BF16, tag="lg_dram")  # 13.6 MB HBM
                sums = persist.tile([128, NCORES * NVC], F32, tag="sums")
                with (
                    tc.tile_pool(name="wv", bufs=2) as wvp,
                    tc.tile_pool(name="exps", bufs=2) as expsp,
                    tc.tile_pool(name="lgc", bufs=3) as lgcp,
                ):
                    for c in range(NVC):
                        cs = slice(c * VCHUNK, (c + 1) * VCHUNK)
                        wv = wvp.tile([128, KT * VCHUNK], BF16, tag="wv")
                        nc.sync.dma_start(
                            out=wv[:].rearrange("p (k n) -> p k n", k=KT),
                            in_=p_owT[:, cs].rearrange("(k p) n -> p k n", p=128),
                        )
                        for r in range(NCORES):
                            ps = gpsum.tile([128, VCHUNK], F32, tag="mmps")
                            for k in range(KT):
                                nc.tensor.matmul(
                                    ps[:],
                                    lhsT=hTall_sb[:, (r * KT + k) * 128 : (r * KT + k + 1) * 128],
                                    rhs=wv[:, k * VCHUNK : (k + 1) * VCHUNK],
                                    start=(k == 0),
                                    stop=False,
                                )
                            nc.tensor.matmul(
                                ps[:], lhsT=ones1b[:], rhs=ob_sb[:1, cs],
                                start=False, stop=True,
                            )
                            lgc = lgcp.tile([128, VCHUNK], BF16, tag="lgc")
                            nc.vector.tensor_copy(out=lgc[:], in_=ps[:])
                            esc = expsp.tile([128, VCHUNK], F32, tag="esc")
                            nc.scalar.activation(
                                out=esc[:],
                                in_=lgc[:],
                                func=ACTF.Exp,
                                accum_out=sums[:, r * NVC + c : r * NVC + c + 1],
                            )
                            nc.sync.dma_start(
                                out=lg_dram[
                                    r * 128 : (r + 1) * 128,
                                    c * VCHUNK : (c + 1) * VCHUNK,
                                ],
                                in_=lgc[:],
                            )

            # ---------- global log-sum-exp + pass B ----------
            if stage >= 6:
                se = persist.tile([128, NCORES], F32, tag="se")
                for r in range(NCORES):
                    nc.vector.reduce_sum(
                        out=se[:, r : r + 1],
                        in_=sums[:, r * NVC : (r + 1) * NVC],
                        axis=AX,
                    )
                se_b = dram.tile([128, NCORES], F32, tag="se_b")
                nc.sync.dma_start(out=se_b[:], in_=se[:])
                se_g = dram.tile([128, NCORES], F32, tag="se_g", addr_space="Shared")
                nc.gpsimd.collective_compute(
                    "AllReduce",
                    ALU.add,
                    ins=[se_b[:].opt()],
                    outs=[se_g[:].opt()],
                    replica_groups=rg,
                )
                seg = persist.tile([128, NCORES], F32, tag="seg")
                nc.sync.dma_start(out=seg[:], in_=se_g[:])
                lse = persist.tile([128, NCORES], F32, tag="lse")
                nc.scalar.activation(out=lse[:], in_=seg[:], func=ACTF.Ln)
                nlse = persist.tile([128, NCORES], F32, tag="nlse")
                nc.vector.tensor_scalar_mul(out=nlse[:], in0=lse[:], scalar1=-1.0)

                with tc.tile_pool(name="ob", bufs=3) as obp:
                    for r in range(NCORES):
                        for c in range(NVC):
                            lgc2 = obp.tile([128, VCHUNK], BF16, tag="lgc2")
                            nc.sync.dma_start(
                                in_=lg_dram[
                                    r * 128 : (r + 1) * 128,
                                    c * VCHUNK : (c + 1) * VCHUNK,
                                ],
                                out=lgc2[:],
                            )
                            obt = obp.tile([128, VCHUNK], F32, tag="obt")
                            nc.vector.tensor_scalar_add(
                                out=obt[:],
                                in0=lgc2[:],
                                scalar1=nlse[:, r : r + 1],
                            )
                            nc.sync.dma_start(
                                out=o_logp[
                                    r * 128 : (r + 1) * 128, c * VCHUNK : (c + 1) * VCHUNK
                                ],
                                in_=obt[:],
                            )
    nc.finalize()
    return nc


_NC_CACHE = None


def _get_nc():
    global _NC_CACHE
    if _NC_CACHE is None:
        _NC_CACHE = build_graph(stage=int(os.environ.get("KSTAGE", "99")))
    return _NC_CACHE


def prepare_in_maps(input_step, last_hidden, encoder_outputs, encoder_mask,
                    emb, w_ih, w_hh, b_ih, b_hh, out_w, out_b):
    input_step = np.asarray(input_step)
    last_hidden = np.asarray(last_hidden, dtype=np.float32)
    encoder_outputs = np.asarray(encoder_outputs, dtype=np.float32)
    encoder_mask = np.asarray(encoder_mask)
    emb = np.ascontiguousarray(np.asarray(emb, dtype=np.float32))
    w_ih = np.asarray(w_ih, dtype=np.float32)
    w_hh = np.asarray(w_hh, dtype=np.float32)
    b_ih = np.asarray(b_ih, dtype=np.float32)
    b_hh = np.asarray(b_hh, dtype=np.float32)
    out_w = np.asarray(out_w, dtype=np.float32)
    out_b = np.asarray(out_b, dtype=np.float32)

    # host-side layout prep (no model math)
    wihT = np.ascontiguousarray(w_ih.T)                      # (H, 3H)
    whhT = np.ascontiguousarray(w_hh.T)
    mask_f = encoder_mask.astype(np.float32)                 # (B, S)
    maskneg = ((mask_f - 1.0) * 1e9).astype(np.float32)
    # padded, transposed, bf16 out_w: (H, 8*VP)
    owT_pad = np.full((H, NCORES * VP), 0.0, dtype=ml_dtypes.bfloat16)
    owT_pad[:, :V] = out_w.T.astype(ml_dtypes.bfloat16)
    ob_pad = np.full((NCORES * VP,), -1e9, dtype=ml_dtypes.bfloat16)
    ob_pad[:V] = out_b.astype(ml_dtypes.bfloat16)

    in_maps = []
    for i in range(NCORES):
        bs = slice(i * BL, (i + 1) * BL)
        vs = slice(i * VP, (i + 1) * VP)
        h0_i = last_hidden[0, bs]                            # (BL, H)
        in_maps.append({
            "idx": input_step[bs].astype(np.int32).reshape(BL, 1),
            "h0": np.ascontiguousarray(h0_i),
            "h0T": np.ascontiguousarray(h0_i.T),
            "enc": np.ascontiguousarray(
                encoder_outputs[bs].reshape(BL, S * H)),
            "mask": np.ascontiguousarray(mask_f[bs]),
            "maskneg": np.ascontiguousarray(maskneg[bs]),
            "w_ihT": wihT,
            "w_hhT": whhT,
            "b_ih": b_ih.reshape(1, G3),
            "b_hh": b_hh.reshape(1, G3),
            "emb": emb,
            "out_wT": np.ascontiguousarray(owT_pad[:, vs]),
            "out_b": np.ascontiguousarray(ob_pad[vs]).reshape(1, VP),
        })

    return in_maps


def assemble_outputs(outs):
    logp = np.concatenate([outs[i]["out_logp"] for i in range(NCORES)], axis=1)
    logp = np.ascontiguousarray(logp[:, :V])
    hidden = np.concatenate([outs[i]["out_h"] for i in range(NCORES)], axis=0)[None]
    attn = np.concatenate([outs[i]["out_attn"] for i in range(NCORES)], axis=0)
    return (logp.astype(np.float32), hidden.astype(np.float32),
            attn.astype(np.float32))


def kernel(**inputs):
    in_maps = prepare_in_maps(**inputs)
    nc = _get_nc()
    res = run_bass_kernel_spmd(nc, in_maps, core_ids=list(range(NCORES)))
    return assemble_outputs(res.results)


if __name__ == "__main__":
    stage = int(os.environ.get("KSTAGE", "99"))
    nc = build_graph(stage=stage)
    print("graph built OK (stage", stage, "):",
          sum(len(bb.instructions) for bb in nc.main_func.blocks), "instructions")


# revision 18
# speedup vs baseline: 6.3338x; 6.3338x over previous
"""AttnDecoderRNN single-step kernel for 8 trn2 NeuronCores.

Strategy (SPMD, one graph, per-core data):
  - batch-parallel front: each core owns 128 of 1024 batch rows.
      embedding gather (indirect DMA from replicated table) -> GRU cell
      (fp32 matmuls + exact DVE/ACT gate math) -> attention over S=10
      encoder positions (DVE fp32).
  - AllGather of the transposed hidden state (bf16) so every core holds
      hT for the full batch.
  - vocab-parallel back: each core owns a 6656-wide padded slice of the
      50257 vocab. logits = hT.T @ out_wT_shard (bf16 matmuls, fp32
      accum), +out_b via K=1 matmul, exp+row-sum fused on ScalarE,
      4KB AllReduce for the global softmax denominator, then
      out = logit - log(sum) written per chunk.
Host only reshapes/transposes/shards/pads and concatenates results.
"""

import os

os.environ.setdefault("MYCRO_LOCAL_CACHE", "1")

import numpy as np
import ml_dtypes

import concourse.bass as bass
import concourse.mybir as mybir
import concourse.tile as tile
from concourse import bacc
from concourse.bass_utils import run_bass_kernel_spmd
from concourse.masks import make_identity

F32 = mybir.dt.float32
F32R = mybir.dt.float32r
BF16 = mybir.dt.bfloat16
I32 = mybir.dt.int32
AX = mybir.AxisListType.X
ALU = mybir.AluOpType
ACTF = mybir.ActivationFunctionType

V = 50257
H = 1024
B = 1024
S = 10
NCORES = 8
BL = B // NCORES          # 128 batch rows per core
VCHUNK = 512              # logits n-chunk (one PSUM bank)
VP = 6656                 # padded vocab per core (13 * 512)
NVC = VP // VCHUNK        # 13
KT = H // 128             # 8 contraction tiles
G3 = 3 * H                # 3072
NGC = G3 // 512           # 6 gate chunks

# matmul dtype knobs
GRU_RDT = F32R            # fp32r: ~1.5e-4 rel err, full PE rate


def build_graph(stage=99):
    """stage gates (debug bisect): 1=gather 2=+gru 3=+attn 4=+allgather
    5=+logitsA 6=full"""
    nc = bacc.Bacc(None, target_bir_lowering=False, debug=False)

    # ---- per-core parameters ----
    p_idx = nc.declare_dram_parameter("idx", [BL, 1], I32, isOutput=False)
    p_h0 = nc.declare_dram_parameter("h0", [BL, H], F32, isOutput=False)
    p_h0T = nc.declare_dram_parameter("h0T", [H, BL], GRU_RDT, isOutput=False)
    p_enc = nc.declare_dram_parameter("enc", [BL, S * H], F32, isOutput=False)
    p_mask = nc.declare_dram_parameter("mask", [BL, S], F32, isOutput=False)
    p_maskneg = nc.declare_dram_parameter("maskneg", [BL, S], F32, isOutput=False)
    p_wihT = nc.declare_dram_parameter("w_ihT", [H, G3], GRU_RDT, isOutput=False)
    p_whhT = nc.declare_dram_parameter("w_hhT", [H, G3], GRU_RDT, isOutput=False)
    p_bih = nc.declare_dram_parameter("b_ih", [1, G3], F32, isOutput=False)
    p_bhh = nc.declare_dram_parameter("b_hh", [1, G3], F32, isOutput=False)
    p_emb = nc.declare_dram_parameter("emb", [V, H], F32, isOutput=False)
    p_owT = nc.declare_dram_parameter("out_wT", [H, VP], BF16, isOutput=False)
    p_ob = nc.declare_dram_parameter("out_b", [1, VP], BF16, isOutput=False)

    o_logp = nc.declare_dram_parameter("out_logp", [B, VP], F32, isOutput=True)
    o_h = nc.declare_dram_parameter("out_h", [BL, H], F32, isOutput=True)
    o_attn = nc.declare_dram_parameter("out_attn", [BL, S], F32, isOutput=True)

    rg = [list(range(NCORES))]

    with tile.TileContext(nc) as tc:
        with (
            tc.tile_pool(name="const", bufs=1) as constp,
            tc.tile_pool(name="persist", bufs=1) as persist,
            tc.tile_pool(name="dram", bufs=1, space="DRAM") as dram,
            tc.tile_pool(name="tpsum", bufs=2, space="PSUM") as tpsum,
            tc.tile_pool(name="gpsum", bufs=4, space="PSUM") as gpsum,
            tc.tile_pool(name="grupool", bufs=1) as grup,
        ):
            # ---------- constants ----------
            ident = constp.tile([128, 128], F32, tag="ident")
            make_identity(nc, ident[:])
            ones1 = constp.tile([1, 128], F32, tag="ones1")
            nc.vector.memset(ones1[:], 1.0)
            ones1b = constp.tile([1, 128], BF16, tag="ones1b")
            nc.vector.memset(ones1b[:], 1.0)

            idx_sb = constp.tile([BL, 1], I32, tag="idx")
            nc.sync.dma_start(out=idx_sb[:], in_=p_idx[:, :])
            bih_sb = constp.tile([1, G3], F32, tag="bih")
            nc.sync.dma_start(out=bih_sb[:], in_=p_bih[:, :])
            bhh_sb = constp.tile([1, G3], F32, tag="bhh")
            nc.sync.dma_start(out=bhh_sb[:], in_=p_bhh[:, :])
            ob_sb = constp.tile([1, VP], BF16, tag="ob")
            nc.sync.dma_start(out=ob_sb[:], in_=p_ob[:, :])
            h0_sb = grup.tile([BL, H], F32, tag="h0")
            nc.sync.dma_start(out=h0_sb[:], in_=p_h0[:, :])
            h0T_sb = grup.tile([128, H], GRU_RDT, tag="h0T")
            nc.sync.dma_start(
                out=h0T_sb[:].rearrange("p (k j) -> p k j", k=KT),
                in_=p_h0T[:, :].rearrange("(k p) j -> p k j", p=128),
            )
            mask_sb = constp.tile([BL, S], F32, tag="mask")
            nc.sync.dma_start(out=mask_sb[:], in_=p_mask[:, :])
            maskneg_sb = constp.tile([BL, S], F32, tag="maskneg")
            nc.sync.dma_start(out=maskneg_sb[:], in_=p_maskneg[:, :])

            # ---------- embedding gather + transpose ----------
            x_sb = grup.tile([BL, H], F32, tag="x")
            nc.gpsimd.indirect_dma_start(
                out=x_sb[:],
                out_offset=None,
                in_=p_emb[:, :],
                in_offset=bass.IndirectOffsetOnAxis(ap=idx_sb[:, :1], axis=0),
            )
            xT_sb = grup.tile([128, H], GRU_RDT, tag="xT")
            for t in range(KT):
                tps = tpsum.tile([128, 128], F32, tag="tps")
                nc.tensor.transpose(tps[:], x_sb[:, t * 128 : (t + 1) * 128], ident[:])
                nc.vector.tensor_copy(out=xT_sb[:, t * 128 : (t + 1) * 128], in_=tps[:])
            if stage == 1:
                nc.sync.dma_start(out=o_h[:, :], in_=x_sb[:])

            # ---------- GRU ----------
            if stage >= 2:
                r_sb = grup.tile([BL, H], F32, tag="r")
                z_sb = grup.tile([BL, H], F32, tag="z")
                n_sb = grup.tile([BL, H], F32, tag="n")
                h_sb = persist.tile([BL, H], F32, tag="h")

                with (
                    tc.tile_pool(name="gw", bufs=2) as gwp,
                    tc.tile_pool(name="gtmp", bufs=3) as gtp,
                ):
                    for c in range(NGC):
                        cs = slice(c * 512, (c + 1) * 512)
                        wih = gwp.tile([128, KT * 512], GRU_RDT, tag="wih")
                        nc.sync.dma_start(
                            out=wih[:].rearrange("p (k n) -> p k n", k=KT),
                            in_=p_wihT[:, cs].rearrange("(k p) n -> p k n", p=128),
                        )
                        whh = gwp.tile([128, KT * 512], GRU_RDT, tag="whh")
                        nc.sync.dma_start(
                            out=whh[:].rearrange("p (k n) -> p k n", k=KT),
                            in_=p_whhT[:, cs].rearrange("(k p) n -> p k n", p=128),
                        )
                        if c < 4:
                            # r/z gates: gi+gh+biases in ONE psum group
                            ps = gpsum.tile([128, 512], F32, tag="mmps")
                            for k in range(KT):
                                nc.tensor.matmul(
                                    ps[:],
                                    lhsT=xT_sb[:, k * 128 : (k + 1) * 128],
                                    rhs=wih[:, k * 512 : (k + 1) * 512],
                                    start=(k == 0),
                                    stop=False,
                                )
                            for k in range(KT):
                                nc.tensor.matmul(
                                    ps[:],
                                    lhsT=h0T_sb[:, k * 128 : (k + 1) * 128],
                                    rhs=whh[:, k * 512 : (k + 1) * 512],
                                    start=False,
                                    stop=False,
                                )
                            nc.tensor.matmul(
                                ps[:], lhsT=ones1[:], rhs=bih_sb[:1, cs],
                                start=False, stop=False,
                            )
                            nc.tensor.matmul(
                                ps[:], lhsT=ones1[:], rhs=bhh_sb[:1, cs],
                                start=False, stop=True,
                            )
                            dst = r_sb if c < 2 else z_sb
                            ds_ = slice((c % 2) * 512, (c % 2) * 512 + 512)
                            nc.scalar.activation(out=dst[:, ds_], in_=ps[:], func=ACTF.Sigmoid)
                        else:
                            # n gate: tanh(gi + r * gh)
                            cc = c - 4
                            ncs = slice(cc * 512, cc * 512 + 512)
                            psi = gpsum.tile([128, 512], F32, tag="mmps")
                            psh = gpsum.tile([128, 512], F32, tag="mmps")
                            for k in range(KT):
                                nc.tensor.matmul(
                                    psi[:],
                                    lhsT=xT_sb[:, k * 128 : (k + 1) * 128],
                                    rhs=wih[:, k * 512 : (k + 1) * 512],
                                    start=(k == 0),
                                    stop=False,
                                )
                            nc.tensor.matmul(
                                psi[:], lhsT=ones1[:], rhs=bih_sb[:1, cs],
                                start=False, stop=True,
                            )
                            for k in range(KT):
                                nc.tensor.matmul(
                                    psh[:],
                                    lhsT=h0T_sb[:, k * 128 : (k + 1) * 128],
                                    rhs=whh[:, k * 512 : (k + 1) * 512],
                                    start=(k == 0),
                                    stop=False,
                                )
                            nc.tensor.matmul(
                                psh[:], lhsT=ones1[:], rhs=bhh_sb[:1, cs],
                                start=False, stop=True,
                            )
                            t1 = gtp.tile([128, 512], F32, tag="t1")
                            nc.vector.tensor_tensor(
                                out=t1[:], in0=r_sb[:, ncs], in1=psh[:], op=ALU.mult
                            )
                            t2 = gtp.tile([128, 512], F32, tag="t2")
                            nc.vector.tensor_tensor(
                                out=t2[:], in0=t1[:], in1=psi[:], op=ALU.add
                            )
                            nc.scalar.activation(out=n_sb[:, ncs], in_=t2[:], func=ACTF.Tanh)

                    # h = n + z * (h0 - n)
                    for q in range(2):
                        qs = slice(q * 512, (q + 1) * 512)
                        d = gtp.tile([128, 512], F32, tag="d")
                        nc.vector.tensor_tensor(
                            out=d[:], in0=h0_sb[:, qs], in1=n_sb[:, qs], op=ALU.subtract
                        )
                        e = gtp.tile([128, 512], F32, tag="e")
                        nc.vector.tensor_tensor(
                            out=e[:], in0=z_sb[:, qs], in1=d[:], op=ALU.mult
                        )
                        nc.vector.tensor_tensor(
                            out=h_sb[:, qs], in0=n_sb[:, qs], in1=e[:], op=ALU.add
                        )

                nc.sync.dma_start(out=o_h[:, :], in_=h_sb[:])

            # ---------- attention (DVE/ACT; overlaps logits PE work) ----------
            if stage >= 3:
                e_sb = persist.tile([BL, S], F32, tag="esb")
                with tc.tile_pool(name="encp", bufs=1) as encp:
                    encs = encp.tile([BL, S * H], F32, tag="encs")
                    nc.sync.dma_start(out=encs[:], in_=p_enc[:, :])
                    prod = encp.tile([BL, S * H], F32, tag="prod")
                    nc.vector.tensor_tensor(
                        out=prod[:].rearrange("p (s j) -> p s j", s=S),
                        in0=h_sb[:].rearrange("p (o j) -> p o j", o=1)
                            .to_broadcast([BL, S, H]),
                        in1=encs[:].rearrange("p (s j) -> p s j", s=S),
                        op=ALU.mult,
                    )
                    nc.vector.reduce_sum(
                        out=e_sb[:].rearrange("p (s o) -> p s o", o=1),
                        in_=prod[:].rearrange("p (s j) -> p s j", s=S),
                        axis=AX,
                    )
                em = persist.tile([BL, S], F32, tag="em")
                nc.vector.tensor_tensor(out=em[:], in0=e_sb[:], in1=mask_sb[:], op=ALU.mult)
                emm = persist.tile([BL, S], F32, tag="emm")
                nc.vector.tensor_tensor(
                    out=emm[:], in0=em[:], in1=maskneg_sb[:], op=ALU.add
                )
                mx = persist.tile([BL, 1], F32, tag="mx")
                nc.vector.reduce_max(out=mx[:, :1], in_=emm[:], axis=AX)
                nmx = persist.tile([BL, 1], F32, tag="nmx")
                nc.vector.tensor_scalar_mul(out=nmx[:, :1], in0=mx[:, :1], scalar1=-1.0)
                ex = persist.tile([BL, S], F32, tag="ex")
                sume = persist.tile([BL, 1], F32, tag="sume")
                nc.scalar.activation(
                    out=ex[:], in_=emm[:], func=ACTF.Exp,
                    bias=nmx[:, :1], accum_out=sume[:, :1],
                )
                rcp = persist.tile([BL, 1], F32, tag="rcp")
                nc.vector.reciprocal(out=rcp[:, :1], in_=sume[:, :1])
                attn_sb = persist.tile([BL, S], F32, tag="attn")
                nc.vector.tensor_scalar_mul(out=attn_sb[:], in0=ex[:], scalar1=rcp[:, :1])
                nc.sync.dma_start(out=o_attn[:, :], in_=attn_sb[:])

            # ---------- hT AllGather ----------
            if stage >= 4:
                hTst = persist.tile([128, H], BF16, tag="hTst")
                for t in range(KT):
                    tps = tpsum.tile([128, 128], F32, tag="tps")
                    nc.tensor.transpose(tps[:], h_sb[:, t * 128 : (t + 1) * 128], ident[:])
                    nc.vector.tensor_copy(out=hTst[:, t * 128 : (t + 1) * 128], in_=tps[:])
                hT_bounce = dram.tile([H, BL], BF16, tag="hT_bounce")
                nc.sync.dma_start(
                    out=hT_bounce[:].rearrange("(t p) j -> p t j", p=128),
                    in_=hTst[:].rearrange("p (t j) -> p t j", t=KT),
                )
                hT_all = dram.tile([NCORES * H, BL], BF16, tag="hT_all", addr_space="Shared")
                nc.gpsimd.collective_compute(
                    "AllGather",
                    ALU.bypass,
                    ins=[hT_bounce[:].opt()],
                    outs=[hT_all[:].opt()],
                    replica_groups=rg,
                )
                hTall_sb = persist.tile([128, NCORES * H], BF16, tag="hTall")
                nc.sync.dma_start(
                    out=hTall_sb[:].rearrange("p (q j) -> p q j", q=NCORES * KT),
                    in_=hT_all[:].rearrange("(q p) j -> p q j", p=128),
                )

            # ---------- logits pass A: matmul, +bias, bf16 store, exp-sum ----------
            if stage >= 5:
                lg_dram = dram.tile([B, VP], BF16, tag="lg_dram")  # 13.6 MB HBM
                sums = persist.tile([128, NCORES * NVC], F32, tag="sums")
                with (
                    tc.tile_pool(name="wv", bufs=3) as wvp,
                    tc.tile_pool(name="exps", bufs=2) as expsp,
                    tc.tile_pool(name="lgc", bufs=3) as lgcp,
                ):
                    for c in range(NVC):
                        cs = slice(c * VCHUNK, (c + 1) * VCHUNK)
                        wv = wvp.tile([128, KT * VCHUNK], BF16, tag="wv")
                        nc.sync.dma_start(
                            out=wv[:].rearrange("p (k n) -> p k n", k=KT),
                            in_=p_owT[:, cs].rearrange("(k p) n -> p k n", p=128),
                        )
                        for r in range(NCORES):
                            ps = gpsum.tile([128, VCHUNK], F32, tag="mmps")
                            for k in range(KT):
                                nc.tensor.matmul(
                                    ps[:],
                                    lhsT=hTall_sb[:, (r * KT + k) * 128 : (r * KT + k + 1) * 128],
                                    rhs=wv[:, k * VCHUNK : (k + 1) * VCHUNK],
                                    start=(k == 0),
                                    stop=False,
                                )
                            nc.tensor.matmul(
                                ps[:], lhsT=ones1b[:], rhs=ob_sb[:1, cs],
                                start=False, stop=True,
                            )
                            lgc = lgcp.tile([128, VCHUNK], BF16, tag="lgc")
                            nc.vector.tensor_copy(out=lgc[:], in_=ps[:])
                            esc = expsp.tile([128, VCHUNK], F32, tag="esc")
                            nc.scalar.activation(
                                out=esc[:],
                                in_=lgc[:],
                                func=ACTF.Exp,
                                accum_out=sums[:, r * NVC + c : r * NVC + c + 1],
                            )
                            nc.sync.dma_start(
                                out=lg_dram[
                                    r * 128 : (r + 1) * 128,
                                    c * VCHUNK : (c + 1) * VCHUNK,
                                ],
                                in_=lgc[:],
                            )

            # ---------- global log-sum-exp + pass B ----------
            if stage >= 6:
                se = persist.tile([128, NCORES], F32, tag="se")
                for r in range(NCORES):
                    nc.vector.reduce_sum(
                        out=se[:, r : r + 1],
                        in_=sums[:, r * NVC : (r + 1) * NVC],
                        axis=AX,
                    )
                se_b = dram.tile([128, NCORES], F32, tag="se_b")
                nc.sync.dma_start(out=se_b[:], in_=se[:])
                se_g = dram.tile([128, NCORES], F32, tag="se_g", addr_space="Shared")
                nc.gpsimd.collective_compute(
                    "AllReduce",
                    ALU.add,
                    ins=[se_b[:].opt()],
                    outs=[se_g[:].opt()],
                    replica_groups=rg,
                )
                seg = persist.tile([128, NCORES], F32, tag="seg")
                nc.sync.dma_start(out=seg[:], in_=se_g[:])
                lse = persist.tile([128, NCORES], F32, tag="lse")
                nc.scalar.activation(out=lse[:], in_=seg[:], func=ACTF.Ln)
                nlse = persist.tile([128, NCORES], F32, tag="nlse")
                nc.vector.tensor_scalar_mul(out=nlse[:], in0=lse[:], scalar1=-1.0)

                with tc.tile_pool(name="ob", bufs=2) as obp:
                    for r in range(NCORES):
                        lgc2 = obp.tile([128, VP], BF16, tag="lgc2")
                        nc.sync.dma_start(
                            in_=lg_dram[r * 128 : (r + 1) * 128, :],
                            out=lgc2[:],
                        )
                        obt = obp.tile([128, VP], F32, tag="obt")
                        nc.vector.tensor_scalar_add(
                            out=obt[:],
                            in0=lgc2[:],
                            scalar1=nlse[:, r : r + 1],
                        )
                        nc.sync.dma_start(
                            out=o_logp[r * 128 : (r + 1) * 128, :],
                            in_=obt[:],
                        )
    nc.finalize()
    return nc


_NC_CACHE = None


def _get_nc():
    global _NC_CACHE
    if _NC_CACHE is None:
        _NC_CACHE = build_graph(stage=int(os.environ.get("KSTAGE", "99")))
    return _NC_CACHE


def prepare_in_maps(input_step, last_hidden, encoder_outputs, encoder_mask,
                    emb, w_ih, w_hh, b_ih, b_hh, out_w, out_b):
    input_step = np.asarray(input_step)
    last_hidden = np.asarray(last_hidden, dtype=np.float32)
    encoder_outputs = np.asarray(encoder_outputs, dtype=np.float32)
    encoder_mask = np.asarray(encoder_mask)
    emb = np.ascontiguousarray(np.asarray(emb, dtype=np.float32))
    w_ih = np.asarray(w_ih, dtype=np.float32)
    w_hh = np.asarray(w_hh, dtype=np.float32)
    b_ih = np.asarray(b_ih, dtype=np.float32)
    b_hh = np.asarray(b_hh, dtype=np.float32)
    out_w = np.asarray(out_w, dtype=np.float32)
    out_b = np.asarray(out_b, dtype=np.float32)

    # host-side layout prep (no model math)
    wihT = np.ascontiguousarray(w_ih.T)                      # (H, 3H)
    whhT = np.ascontiguousarray(w_hh.T)
    mask_f = encoder_mask.astype(np.float32)                 # (B, S)
    maskneg = ((mask_f - 1.0) * 1e9).astype(np.float32)
    # padded, transposed, bf16 out_w: (H, 8*VP)
    owT_pad = np.full((H, NCORES * VP), 0.0, dtype=ml_dtypes.bfloat16)
    owT_pad[:, :V] = out_w.T.astype(ml_dtypes.bfloat16)
    ob_pad = np.full((NCORES * VP,), -1e9, dtype=ml_dtypes.bfloat16)
    ob_pad[:V] = out_b.astype(ml_dtypes.bfloat16)

    in_maps = []
    for i in range(NCORES):
        bs = slice(i * BL, (i + 1) * BL)
        vs = slice(i * VP, (i + 1) * VP)
        h0_i = last_hidden[0, bs]                            # (BL, H)
        in_maps.append({
            "idx": input_step[bs].astype(np.int32).reshape(BL, 1),
            "h0": np.ascontiguousarray(h0_i),
            "h0T": np.ascontiguousarray(h0_i.T),
            "enc": np.ascontiguousarray(
                encoder_outputs[bs].reshape(BL, S * H)),
            "mask": np.ascontiguousarray(mask_f[bs]),
            "maskneg": np.ascontiguousarray(maskneg[bs]),
            "w_ihT": wihT,
            "w_hhT": whhT,
            "b_ih": b_ih.reshape(1, G3),
            "b_hh": b_hh.reshape(1, G3),
            "emb": emb,
            "out_wT": np.ascontiguousarray(owT_pad[:, vs]),
            "out_b": np.ascontiguousarray(ob_pad[vs]).reshape(1, VP),
        })

    return in_maps


def assemble_outputs(outs):
    logp = np.concatenate([outs[i]["out_logp"] for i in range(NCORES)], axis=1)
    logp = np.ascontiguousarray(logp[:, :V])
    hidden = np.concatenate([outs[i]["out_h"] for i in range(NCORES)], axis=0)[None]
    attn = np.concatenate([outs[i]["out_attn"] for i in range(NCORES)], axis=0)
    return (logp.astype(np.float32), hidden.astype(np.float32),
            attn.astype(np.float32))


def kernel(**inputs):
    in_maps = prepare_in_maps(**inputs)
    nc = _get_nc()
    res = run_bass_kernel_spmd(nc, in_maps, core_ids=list(range(NCORES)))
    return assemble_outputs(res.results)


if __name__ == "__main__":
    stage = int(os.environ.get("KSTAGE", "99"))
    nc = build_graph(stage=stage)
    print("graph built OK (stage", stage, "):",
          sum(len(bb.instructions) for bb in nc.main_func.blocks), "instructions")
